# revision 17
# baseline (speedup 1.0000x reference)
"""Trainium2 Bass kernel for nn_AttnApproximator (B=2, S=1024, H=4096, NH=32, HD=128).

Sharding: 8 cores = (batch 2) x (4 head-groups of 8 heads).  Per core:
Q/K/V projections for its 8 heads (fp32r matmuls), RoPE, transposed scores
[kv, q] per head (so exp / Z2 / denominators / attn@V need no on-chip
transposes; reductions over kv become ones-matmuls that also broadcast),
causal masking via a sliding mask multiply, and a row-parallel o_proj
partial.  Host sums the 4 partials per batch element and the Z2 partials.
"""

import numpy as np

NH, HD, S, H, B = 32, 128, 1024, 4096, 2
NCORES = 8
GROUPS = 4            # head-groups (tensor parallel)
HPG = NH // GROUPS    # 8 heads per core
DC = HPG * HD         # 1024 per-core output dims
HPP = 4               # heads per pass
NPASS = HPG // HPP    # 2
ROPE_BASE = 10000.0
ISQ = float(1.0 / np.sqrt(HD))

_CACHE = {}


def _build_program():
    import concourse.bacc as bacc
    import concourse.mybir as mybir
    import concourse.tile as tile

    F32 = mybir.dt.float32
    F32R = mybir.dt.float32r
    AF = mybir.ActivationFunctionType

    nc = bacc.Bacc("TRN2", target_bir_lowering=False, debug=False)
    xT = nc.dram_tensor("xT", [H, S], F32R, kind="ExternalInput")      # (h, s)
    # weights host-pre-tiled so every streamed tile is one contiguous block
    wqT = nc.dram_tensor("wqT", [NPASS, 32, 128, 512], F32R, kind="ExternalInput")
    wkT = nc.dram_tensor("wkT", [NPASS, 32, 128, 512], F32R, kind="ExternalInput")
    wvT = nc.dram_tensor("wvT", [NPASS, 32, 128, 512], F32R, kind="ExternalInput")
    woT = nc.dram_tensor("woT", [8, 128, 8, 512], F32R, kind="ExternalInput")
    cosT = nc.dram_tensor("cosT", [HD, S], F32, kind="ExternalInput")
    ssinT = nc.dram_tensor("ssinT", [HD, S], F32, kind="ExternalInput")
    onesD = nc.dram_tensor("onesD", [128, 128], F32R, kind="ExternalInput")
    maskD = nc.dram_tensor("maskD", [128, 2 * S], F32R, kind="ExternalInput")
    outD = nc.dram_tensor("out", [S, H], F32, kind="ExternalOutput")
    z2D = nc.dram_tensor("z2", [1, S], F32, kind="ExternalOutput")

    KT_TILES = H // 128   # 32
    KH = 2                # k-halves (PSUM-capacity spill granularity)
    KPH = KT_TILES // KH  # 16
    PW = HPP * HD         # 512 projection columns per pass

    with tile.TileContext(nc) as tc:
        with (
            tc.tile_pool(name="const", bufs=1) as constp,
            tc.tile_pool(name="persist", bufs=1) as persist,
        ):
            ones = constp.tile([128, 128], F32R, tag="ones")
            nc.sync.dma_start(ones[:], onesD[:])
            cos_t = constp.tile([HD, S], F32, tag="cos")
            nc.sync.dma_start(cos_t[:], cosT[:])
            ssin_t = constp.tile([HD, S], F32, tag="ssin")
            nc.sync.dma_start(ssin_t[:], ssinT[:])
            mask_t = constp.tile([128, 2 * S], F32R, tag="mask")
            nc.sync.dma_start(mask_t[:], maskD[:])

            z2acc = persist.tile([128, S], F32, tag="z2acc")
            ctxt = persist.tile([128, HPG, S], F32R, tag="ctxt")  # (d, head, q)

            with tc.tile_pool(name="xroll", bufs=20) as xp:
              for p in range(NPASS):
                dlo = p * PW
                with tc.tile_pool(name=f"qkv{p}", bufs=1) as qkvp:
                    QT = qkvp.tile([128, HPP, S], F32R, tag="QT")   # (d, head, s)
                    KTt = qkvp.tile([128, HPP, S], F32R, tag="KT")
                    V = qkvp.tile([128, 8, PW], F32R, tag="V")      # (s%128, s//128, d)

                    # ---------------- phase A: projections + RoPE ------------
                    with (
                        tc.tile_pool(name=f"wroll{p}", bufs=7) as wp,
                        tc.tile_pool(name=f"tmp{p}", bufs=1) as tmpp,
                    ):
                        def issue_x(kk, nsplit=2):
                            xt = xp.tile([128, S], F32R, tag="x", name=f"x{kk}")
                            step = 128 // nsplit
                            for q in range(nsplit):
                                nc.sync.dma_start(
                                    xt[q * step:(q + 1) * step, :],
                                    xT[kk * 128 + q * step:
                                       kk * 128 + (q + 1) * step, :])
                            return xt

                        for kh in range(KH):
                            xts = [issue_x(kh * KPH + k, nsplit=4)
                                   for k in range(2)]
                            with tc.tile_pool(name=f"psqk{p}{kh}", bufs=4,
                                              space="PSUM") as psqk:
                                # Q then K (4 psum targets of [128, S] = 8 banks)
                                for wi, (W, dst) in enumerate(((wqT, QT), (wkT, KTt))):
                                    pss = [psqk.tile([128, S], F32, tag="pqk", name=f"pqk{_i}")
                                           for _i in range(HPP)]
                                    for k in range(KPH):
                                        kk = kh * KPH + k
                                        if wi == 0 and k + 2 < KPH:
                                            xts.append(issue_x(kh * KPH + k + 2))
                                        wt = wp.tile([128, PW], F32R, tag="w")
                                        nc.sync.dma_start(wt[:], W[p, kk])
                                        for m in range(HPP):
                                            for g in range(2):
                                                nc.tensor.matmul(
                                                    pss[m][:, g * 512:(g + 1) * 512],
                                                    wt[:, m * 128:(m + 1) * 128],
                                                    xts[k][:, g * 512:(g + 1) * 512],
                                                    start=(k == 0), stop=(k == KPH - 1))
                                    for m in range(HPP):
                                        if kh == 0:
                                            nc.vector.tensor_copy(dst[:, m, :], pss[m][:])
                                        else:
                                            nc.vector.tensor_add(dst[:, m, :], pss[m][:],
                                                                 dst[:, m, :])
                                # V: 8 targets of [128, 512], packed in pairs into
                                # the same 4-slot [128, S] pool (same banks)
                                pvs = [psqk.tile([128, S], F32, tag="pqk", name=f"pv{_i}")
                                       for _i in range(4)]
                                for k in range(KPH):
                                    kk = kh * KPH + k
                                    wt = wp.tile([128, PW], F32R, tag="w")
                                    nc.sync.dma_start(wt[:], wvT[p, kk])
                                    for st in range(8):
                                        nc.tensor.matmul(
                                            pvs[st // 2][:, (st % 2) * 512:
                                                         (st % 2) * 512 + 512],
                                            xts[k][:, st * 128:(st + 1) * 128],
                                            wt[:],
                                            start=(k == 0), stop=(k == KPH - 1))
                                for st in range(8):
                                    vsrc = pvs[st // 2][:, (st % 2) * 512:
                                                        (st % 2) * 512 + 512]
                                    if kh == 0:
                                        nc.vector.tensor_copy(V[:, st, :], vsrc)
                                    else:
                                        nc.vector.tensor_add(V[:, st, :], vsrc,
                                                             V[:, st, :])
                        # RoPE: dst = dst*cos + swap_halves(dst)*ssin
                        for dst in (QT, KTt):
                            for m in range(HPP):
                                t1 = tmpp.tile([128, S], F32, tag="t1")
                                sh = tmpp.tile([128, S], F32R, tag="sh")
                                nc.sync.dma_start(sh[0:64, :], dst[64:128, m, :])
                                nc.sync.dma_start(sh[64:128, :], dst[0:64, m, :])
                                nc.vector.tensor_mul(t1[:], dst[:, m, :], cos_t[:])
                                nc.gpsimd.tensor_mul(sh[:], sh[:], ssin_t[:])
                                nc.vector.tensor_add(dst[:, m, :], t1[:], sh[:])

                    # ---------------- phase B: attention ---------------------
                    with (
                        tc.tile_pool(name=f"et{p}", bufs=4) as etp,
                        tc.tile_pool(name=f"rec{p}", bufs=2) as recp,
                        tc.tile_pool(name=f"pss{p}", bufs=2, space="PSUM") as pssc,
                        tc.tile_pool(name=f"psd{p}", bufs=1, space="PSUM") as psdp,
                        tc.tile_pool(name=f"psc{p}", bufs=1, space="PSUM") as pscp,
                        tc.tile_pool(name=f"psz{p}", bufs=1, space="PSUM") as pszp,
                    ):
                        z2ps = pszp.tile([128, S], F32, tag="z2")
                        for m in range(HPP):
                            den = psdp.tile([128, S], F32, tag="den")
                            ctx = pscp.tile([128, S], F32, tag="ctx")
                            for t in range(8):
                                et_t = etp.tile([128, S], F32R, tag="et",
                                                name="et_t")  # (kv, q)
                                for g in range(2):
                                    sps = pssc.tile([128, 512], F32, tag="s")
                                    nc.tensor.matmul(
                                        sps[:],
                                        KTt[:, m, t * 128:(t + 1) * 128],
                                        QT[:, m, g * 512:(g + 1) * 512],
                                        start=True, stop=True)
                                    nc.scalar.activation(
                                        et_t[:, g * 512:(g + 1) * 512], sps[:],
                                        AF.Exp, scale=ISQ)
                                for g in range(2):
                                    nc.tensor.matmul(
                                        z2ps[:, g * 512:(g + 1) * 512],
                                        ones[:],
                                        et_t[:, g * 512:(g + 1) * 512],
                                        start=(m == 0 and t == 0),
                                        stop=(m == HPP - 1 and t == 7))
                                # causal mask: keep where q >= kv_abs
                                nc.vector.tensor_mul(
                                    et_t[:],  et_t[:],
                                    mask_t[:, S - t * 128:2 * S - t * 128])
                                for g in range(2):
                                    nc.tensor.matmul(
                                        den[:, g * 512:(g + 1) * 512],
                                        ones[:],
                                        et_t[:, g * 512:(g + 1) * 512],
                                        start=(t == 0), stop=(t == 7))
                                for g in range(2):
                                    nc.tensor.matmul(
                                        ctx[:, g * 512:(g + 1) * 512],
                                        V[:, t, m * 128:(m + 1) * 128],
                                        et_t[:, g * 512:(g + 1) * 512],
                                        start=(t == 0), stop=(t == 7))
                            rec = recp.tile([128, S], F32, tag="rec")
                            nc.vector.reciprocal_approx_fast(out=rec[:], in_=den[:])
                            hh = p * HPP + m
                            nc.vector.tensor_mul(ctxt[:, hh, :], ctx[:], rec[:])
                        if p == 0:
                            nc.vector.tensor_copy(z2acc[:], z2ps[:])
                        else:
                            nc.vector.tensor_add(z2acc[:], z2ps[:], z2acc[:])

              # ---------------- phase C: row-parallel o_proj -----------------
              with (
                  tc.tile_pool(name="wo", bufs=2) as wop,
                  tc.tile_pool(name="ostg", bufs=3) as ostgp,
                  tc.tile_pool(name="pso", bufs=4, space="PSUM") as psop,
              ):
                  for jq in range(8):
                      wo = wop.tile([128, HPG, 512], F32R, tag="wo")
                      nc.sync.dma_start(wo[0:64], woT[jq, 0:64])
                      nc.sync.dma_start(wo[64:128], woT[jq, 64:128])
                      for st in range(8):
                          ps = psop.tile([128, 512], F32, tag="po")
                          for i in range(HPG):
                              nc.tensor.matmul(
                                  ps[:],
                                  ctxt[:, i, st * 128:(st + 1) * 128],
                                  wo[:, i, :],
                                  start=(i == 0), stop=(i == HPG - 1))
                          stg = ostgp.tile([128, 512], F32, tag="stg")
                          nc.vector.tensor_copy(stg[:], ps[:])
                          nc.sync.dma_start(
                              outD[st * 128:(st + 1) * 128,
                                   jq * 512:(jq + 1) * 512], stg[:])

            nc.sync.dma_start(z2D[:], z2acc[0:1, :])

    nc.compile()
    return nc


def _rope_tables_np(position_ids_b):
    inv_freq = 1.0 / (ROPE_BASE ** (np.arange(0, HD, 2, dtype=np.float32) / HD))
    t = position_ids_b.astype(np.float32)
    freqs = np.outer(t, inv_freq)                      # [S, HD/2]
    emb = np.concatenate([freqs, freqs], axis=-1)      # [S, HD]
    cosT = np.ascontiguousarray(np.cos(emb).T, dtype=np.float32)   # [HD, S]
    sinT = np.cos(0)  # placeholder, replaced below
    sinT = np.ascontiguousarray(np.sin(emb).T, dtype=np.float32)
    ssinT = sinT.copy()
    ssinT[: HD // 2] = -ssinT[: HD // 2]
    return cosT, ssinT


def _causal_mask_np():
    # maskbig[p, u] = 1 iff u >= S + p ; slice [S - t*128 : 2S - t*128] gives
    # keep(q, kv=t*128+p) = q >= t*128 + p
    p = np.arange(128)[:, None]
    u = np.arange(2 * S)[None, :]
    return (u >= S + p).astype(np.float32)


def run(inputs, trace=False):
    from concourse import bass_utils

    hidden_states = np.asarray(inputs["hidden_states"], dtype=np.float32)
    Wq = np.asarray(inputs["Wq"], dtype=np.float32)
    Wk = np.asarray(inputs["Wk"], dtype=np.float32)
    Wv = np.asarray(inputs["Wv"], dtype=np.float32)
    Wo = np.asarray(inputs["Wo"], dtype=np.float32)
    position_ids = np.asarray(inputs["position_ids"])

    if "nc" not in _CACHE:
        _CACHE["nc"] = _build_program()
    nc = _CACHE["nc"]

    ones = np.ones((128, 128), dtype=np.float32)
    mask = _causal_mask_np()

    def tile_w(W_rows_T):
        # [H, DC] (h, d) -> [NPASS, 32, 128, 512] contiguous k-tiles
        return np.ascontiguousarray(
            W_rows_T.reshape(32, 128, NPASS, 512).transpose(2, 0, 1, 3))

    def tile_wo(WoT):
        # [DC, H] (i, j) -> [8 jq, 128 p, 8 t, 512 j'] contiguous chunks
        return np.ascontiguousarray(
            WoT.reshape(8, 128, 8, 512).transpose(2, 1, 0, 3))

    in_maps = []
    for c in range(NCORES):
        b, g = c // GROUPS, c % GROUPS
        rows = slice(g * DC, (g + 1) * DC)
        cosT, ssinT = _rope_tables_np(position_ids[b])
        in_maps.append({
            "xT": np.ascontiguousarray(hidden_states[b].T),
            "wqT": tile_w(Wq[rows, :].T),
            "wkT": tile_w(Wk[rows, :].T),
            "wvT": tile_w(Wv[rows, :].T),
            "woT": tile_wo(Wo[:, rows].T),
            "cosT": cosT,
            "ssinT": ssinT,
            "onesD": ones,
            "maskD": mask,
        })

    res = bass_utils.run_bass_kernel_spmd(
        nc, in_maps, core_ids=list(range(NCORES)), trace=trace)

    out = np.zeros((B, S, H), dtype=np.float32)
    Z2 = np.zeros((B, S), dtype=np.float32)
    for c in range(NCORES):
        b = c // GROUPS
        out[b] += res.results[c]["out"]
        Z2[b] += res.results[c]["z2"][0]
    Z2 /= NH
    return (out, Z2), res


def kernel(**inputs):
    (out, Z2), _ = run(inputs, trace=False)
    return out, Z2


# revision 24
# speedup vs baseline: 1.0930x; 1.0930x over previous
"""Trainium2 Bass kernel for nn_AttnApproximator (B=2, S=1024, H=4096, NH=32, HD=128).

Sharding: 8 cores = (batch 2) x (4 head-groups of 8 heads).  Per core:
Q/K/V projections for its 8 heads (fp32r matmuls), RoPE, transposed scores
[kv, q] per head (so exp / Z2 / denominators / attn@V need no on-chip
transposes; reductions over kv become ones-matmuls that also broadcast),
causal masking via a sliding mask multiply, and a row-parallel o_proj
partial.  Host sums the 4 partials per batch element and the Z2 partials.
"""

import numpy as np

NH, HD, S, H, B = 32, 128, 1024, 4096, 2
NCORES = 8
GROUPS = 4            # head-groups (tensor parallel)
HPG = NH // GROUPS    # 8 heads per core
DC = HPG * HD         # 1024 per-core output dims
HPP = 4               # heads per pass
NPASS = HPG // HPP    # 2
ROPE_BASE = 10000.0
ISQ = float(1.0 / np.sqrt(HD))

_CACHE = {}


def _build_program():
    import concourse.bacc as bacc
    import concourse.mybir as mybir
    import concourse.tile as tile

    F32 = mybir.dt.float32
    F32R = mybir.dt.float32r
    AF = mybir.ActivationFunctionType

    nc = bacc.Bacc("TRN2", target_bir_lowering=False, debug=False)
    xT = nc.dram_tensor("xT", [H, S], F32R, kind="ExternalInput")      # (h, s)
    # weights host-pre-tiled so every streamed tile is one contiguous block
    wqT = nc.dram_tensor("wqT", [NPASS, 32, 128, 512], F32R, kind="ExternalInput")
    wkT = nc.dram_tensor("wkT", [NPASS, 32, 128, 512], F32R, kind="ExternalInput")
    wvT = nc.dram_tensor("wvT", [NPASS, 32, 128, 512], F32R, kind="ExternalInput")
    woT = nc.dram_tensor("woT", [8, 128, 8, 512], F32R, kind="ExternalInput")
    cosT = nc.dram_tensor("cosT", [HD, S], F32, kind="ExternalInput")
    ssinT = nc.dram_tensor("ssinT", [HD, S], F32, kind="ExternalInput")
    onesD = nc.dram_tensor("onesD", [128, 128], F32R, kind="ExternalInput")
    maskD = nc.dram_tensor("maskD", [128, 128], F32R, kind="ExternalInput")
    outD = nc.dram_tensor("out", [S, H], F32, kind="ExternalOutput")
    z2D = nc.dram_tensor("z2", [1, S], F32, kind="ExternalOutput")

    KT_TILES = H // 128   # 32
    KH = 2                # k-halves (PSUM-capacity spill granularity)
    KPH = KT_TILES // KH  # 16
    PW = HPP * HD         # 512 projection columns per pass

    with tile.TileContext(nc) as tc:
        with (
            tc.tile_pool(name="const", bufs=1) as constp,
            tc.tile_pool(name="persist", bufs=1) as persist,
        ):
            ones = constp.tile([128, 128], F32R, tag="ones")
            cos_t = constp.tile([HD, S], F32, tag="cos")
            ssin_t = constp.tile([HD, S], F32, tag="ssin")
            mask_t = constp.tile([128, 128], F32R, tag="mask")

            def issue_consts():
                nc.sync.dma_start(ones[:], onesD[:])
                nc.sync.dma_start(cos_t[:], cosT[:])
                nc.sync.dma_start(ssin_t[:], ssinT[:])
                nc.sync.dma_start(mask_t[:], maskD[:])

            z2acc = persist.tile([128, S], F32, tag="z2acc")
            ctxt = persist.tile([128, HPG, S], F32R, tag="ctxt")  # (d, head, q)

            with tc.tile_pool(name="xroll", bufs=20) as xp:
              for p in range(NPASS):
                dlo = p * PW
                with tc.tile_pool(name=f"qkv{p}", bufs=1) as qkvp:
                    QT = qkvp.tile([128, HPP, S], F32R, tag="QT")   # (d, head, s)
                    KTt = qkvp.tile([128, HPP, S], F32R, tag="KT")
                    V = qkvp.tile([128, 8, PW], F32R, tag="V")      # (s%128, s//128, d)

                    # ---------------- phase A: projections + RoPE ------------
                    with (
                        tc.tile_pool(name=f"wroll{p}", bufs=7) as wp,
                        tc.tile_pool(name=f"tmp{p}", bufs=2) as tmpp,
                    ):
                        def issue_x(kk, nsplit=2):
                            xt = xp.tile([128, S], F32R, tag="x", name=f"x{kk}")
                            step = 128 // nsplit
                            for q in range(nsplit):
                                nc.sync.dma_start(
                                    xt[q * step:(q + 1) * step, :],
                                    xT[kk * 128 + q * step:
                                       kk * 128 + (q + 1) * step, :])
                            return xt

                        for kh in range(KH):
                            xts = [issue_x(kh * KPH + k, nsplit=4)
                                   for k in range(2)]
                            if p == 0 and kh == 0:
                                issue_consts()
                            with tc.tile_pool(name=f"psqk{p}{kh}", bufs=4,
                                              space="PSUM") as psqk:
                                # Q then K (4 psum targets of [128, S] = 8 banks)
                                for wi, (W, dst) in enumerate(((wqT, QT), (wkT, KTt))):
                                    pss = [psqk.tile([128, S], F32, tag="pqk", name=f"pqk{_i}")
                                           for _i in range(HPP)]
                                    for k in range(KPH):
                                        kk = kh * KPH + k
                                        if wi == 0 and k + 2 < KPH:
                                            xts.append(issue_x(kh * KPH + k + 2))
                                        wt = wp.tile([128, PW], F32R, tag="w")
                                        nc.sync.dma_start(wt[:], W[p, kk])
                                        for m in range(HPP):
                                            for g in range(2):
                                                nc.tensor.matmul(
                                                    pss[m][:, g * 512:(g + 1) * 512],
                                                    wt[:, m * 128:(m + 1) * 128],
                                                    xts[k][:, g * 512:(g + 1) * 512],
                                                    start=(k == 0), stop=(k == KPH - 1))
                                    for m in range(HPP):
                                        if kh == 0:
                                            nc.vector.tensor_copy(dst[:, m, :], pss[m][:])
                                        else:
                                            nc.vector.tensor_add(dst[:, m, :], pss[m][:],
                                                                 dst[:, m, :])
                                # V: 8 targets of [128, 512], packed in pairs into
                                # the same 4-slot [128, S] pool (same banks)
                                pvs = [psqk.tile([128, S], F32, tag="pqk", name=f"pv{_i}")
                                       for _i in range(4)]
                                for k in range(KPH):
                                    kk = kh * KPH + k
                                    wt = wp.tile([128, PW], F32R, tag="w")
                                    nc.sync.dma_start(wt[:], wvT[p, kk])
                                    for st in range(8):
                                        nc.tensor.matmul(
                                            pvs[st // 2][:, (st % 2) * 512:
                                                         (st % 2) * 512 + 512],
                                            xts[k][:, st * 128:(st + 1) * 128],
                                            wt[:],
                                            start=(k == 0), stop=(k == KPH - 1))
                                for st in range(8):
                                    vsrc = pvs[st // 2][:, (st % 2) * 512:
                                                        (st % 2) * 512 + 512]
                                    if kh == 0:
                                        nc.vector.tensor_copy(V[:, st, :], vsrc)
                                    else:
                                        nc.vector.tensor_add(V[:, st, :], vsrc,
                                                             V[:, st, :])
                        # RoPE: dst = dst*cos + swap_halves(dst)*ssin
                        for dst in (QT, KTt):
                            for m in range(HPP):
                                t1 = tmpp.tile([128, S], F32, tag="t1")
                                sh = tmpp.tile([128, S], F32R, tag="sh")
                                nc.sync.dma_start(sh[0:64, :], dst[64:128, m, :])
                                nc.sync.dma_start(sh[64:128, :], dst[0:64, m, :])
                                nc.vector.tensor_mul(t1[:], dst[:, m, :], cos_t[:])
                                nc.vector.tensor_mul(sh[:], sh[:], ssin_t[:])
                                nc.vector.tensor_add(dst[:, m, :], t1[:], sh[:])

                    # ---------------- phase B: attention ---------------------
                    with (
                        tc.tile_pool(name=f"et{p}", bufs=4) as etp,
                        tc.tile_pool(name=f"rec{p}", bufs=2) as recp,
                        tc.tile_pool(name=f"pss{p}", bufs=2, space="PSUM") as pssc,
                        tc.tile_pool(name=f"psd{p}", bufs=1, space="PSUM") as psdp,
                        tc.tile_pool(name=f"psc{p}", bufs=1, space="PSUM") as pscp,
                        tc.tile_pool(name=f"psz{p}", bufs=1, space="PSUM") as pszp,
                    ):
                        z2ps = pszp.tile([128, S], F32, tag="z2")
                        for m in range(HPP):
                            den = psdp.tile([128, S], F32, tag="den")
                            ctx = pscp.tile([128, S], F32, tag="ctx")
                            for t in range(8):
                                et_t = etp.tile([128, S], F32R, tag="et",
                                                name="et_t")  # (kv, q)
                                for g in range(2):
                                    sps = pssc.tile([128, 512], F32, tag="s")
                                    nc.tensor.matmul(
                                        sps[:],
                                        KTt[:, m, t * 128:(t + 1) * 128],
                                        QT[:, m, g * 512:(g + 1) * 512],
                                        start=True, stop=True)
                                    nc.scalar.activation(
                                        et_t[:, g * 512:(g + 1) * 512], sps[:],
                                        AF.Exp, scale=ISQ)
                                for g in range(2):
                                    nc.tensor.matmul(
                                        z2ps[:, g * 512:(g + 1) * 512],
                                        ones[:],
                                        et_t[:, g * 512:(g + 1) * 512],
                                        start=(m == 0 and t == 0),
                                        stop=(m == HPP - 1 and t == 7))
                                # causal: only the diagonal block needs masking;
                                # q-columns < t*128 are simply not read below.
                                nc.vector.tensor_mul(
                                    et_t[:, t * 128:(t + 1) * 128],
                                    et_t[:, t * 128:(t + 1) * 128],
                                    mask_t[:])
                                for g in range(2):
                                    lo, hi = max(t * 128, g * 512), (g + 1) * 512
                                    if lo >= hi:
                                        continue
                                    stop_t = 3 if g == 0 else 7
                                    nc.tensor.matmul(
                                        den[:, lo:hi],
                                        ones[:],
                                        et_t[:, lo:hi],
                                        start=(t == 0), stop=(t == stop_t),
                                        skip_group_check=True)
                                for g in range(2):
                                    lo, hi = max(t * 128, g * 512), (g + 1) * 512
                                    if lo >= hi:
                                        continue
                                    stop_t = 3 if g == 0 else 7
                                    nc.tensor.matmul(
                                        ctx[:, lo:hi],
                                        V[:, t, m * 128:(m + 1) * 128],
                                        et_t[:, lo:hi],
                                        start=(t == 0), stop=(t == stop_t),
                                        skip_group_check=True)
                            rec = recp.tile([128, S], F32, tag="rec")
                            nc.vector.reciprocal_approx_fast(out=rec[:], in_=den[:])
                            hh = p * HPP + m
                            nc.vector.tensor_mul(ctxt[:, hh, :], ctx[:], rec[:])
                        if p == 0:
                            nc.vector.tensor_copy(z2acc[:], z2ps[:])
                        else:
                            nc.vector.tensor_add(z2acc[:], z2ps[:], z2acc[:])

              # ---------------- phase C: row-parallel o_proj -----------------
              with (
                  tc.tile_pool(name="wo", bufs=2) as wop,
                  tc.tile_pool(name="ostg", bufs=3) as ostgp,
                  tc.tile_pool(name="pso", bufs=4, space="PSUM") as psop,
              ):
                  for jq in range(8):
                      wo = wop.tile([128, HPG, 512], F32R, tag="wo")
                      nc.sync.dma_start(wo[0:64], woT[jq, 0:64])
                      nc.sync.dma_start(wo[64:128], woT[jq, 64:128])
                      for st in range(8):
                          ps = psop.tile([128, 512], F32, tag="po")
                          for i in range(HPG):
                              nc.tensor.matmul(
                                  ps[:],
                                  ctxt[:, i, st * 128:(st + 1) * 128],
                                  wo[:, i, :],
                                  start=(i == 0), stop=(i == HPG - 1))
                          stg = ostgp.tile([128, 512], F32, tag="stg")
                          nc.vector.tensor_copy(stg[:], ps[:])
                          nc.sync.dma_start(
                              outD[st * 128:(st + 1) * 128,
                                   jq * 512:(jq + 1) * 512], stg[:])

            nc.sync.dma_start(z2D[:], z2acc[0:1, :])

    nc.compile()
    return nc


def _rope_tables_np(position_ids_b):
    inv_freq = 1.0 / (ROPE_BASE ** (np.arange(0, HD, 2, dtype=np.float32) / HD))
    t = position_ids_b.astype(np.float32)
    freqs = np.outer(t, inv_freq)                      # [S, HD/2]
    emb = np.concatenate([freqs, freqs], axis=-1)      # [S, HD]
    cosT = np.ascontiguousarray(np.cos(emb).T, dtype=np.float32)   # [HD, S]
    sinT = np.cos(0)  # placeholder, replaced below
    sinT = np.ascontiguousarray(np.sin(emb).T, dtype=np.float32)
    ssinT = sinT.copy()
    ssinT[: HD // 2] = -ssinT[: HD // 2]
    return cosT, ssinT


def _causal_mask_np():
    # diagonal-block mask: keep(p, r) = r >= p (kv = t*128+p, q = t*128+r)
    p = np.arange(128)[:, None]
    r = np.arange(128)[None, :]
    return (r >= p).astype(np.float32)


def run(inputs, trace=False):
    from concourse import bass_utils

    hidden_states = np.asarray(inputs["hidden_states"], dtype=np.float32)
    Wq = np.asarray(inputs["Wq"], dtype=np.float32)
    Wk = np.asarray(inputs["Wk"], dtype=np.float32)
    Wv = np.asarray(inputs["Wv"], dtype=np.float32)
    Wo = np.asarray(inputs["Wo"], dtype=np.float32)
    position_ids = np.asarray(inputs["position_ids"])

    if "nc" not in _CACHE:
        _CACHE["nc"] = _build_program()
    nc = _CACHE["nc"]

    ones = np.ones((128, 128), dtype=np.float32)
    mask = _causal_mask_np()

    def tile_w(W_rows_T):
        # [H, DC] (h, d) -> [NPASS, 32, 128, 512] contiguous k-tiles
        return np.ascontiguousarray(
            W_rows_T.reshape(32, 128, NPASS, 512).transpose(2, 0, 1, 3))

    def tile_wo(WoT):
        # [DC, H] (i, j) -> [8 jq, 128 p, 8 t, 512 j'] contiguous chunks
        return np.ascontiguousarray(
            WoT.reshape(8, 128, 8, 512).transpose(2, 1, 0, 3))

    in_maps = []
    for c in range(NCORES):
        b, g = c // GROUPS, c % GROUPS
        rows = slice(g * DC, (g + 1) * DC)
        cosT, ssinT = _rope_tables_np(position_ids[b])
        in_maps.append({
            "xT": np.ascontiguousarray(hidden_states[b].T),
            "wqT": tile_w(Wq[rows, :].T),
            "wkT": tile_w(Wk[rows, :].T),
            "wvT": tile_w(Wv[rows, :].T),
            "woT": tile_wo(Wo[:, rows].T),
            "cosT": cosT,
            "ssinT": ssinT,
            "onesD": ones,
            "maskD": mask,
        })

    res = bass_utils.run_bass_kernel_spmd(
        nc, in_maps, core_ids=list(range(NCORES)), trace=trace)

    out = np.zeros((B, S, H), dtype=np.float32)
    Z2 = np.zeros((B, S), dtype=np.float32)
    for c in range(NCORES):
        b = c // GROUPS
        out[b] += res.results[c]["out"]
        Z2[b] += res.results[c]["z2"][0]
    Z2 /= NH
    return (out, Z2), res


def kernel(**inputs):
    (out, Z2), _ = run(inputs, trace=False)
    return out, Z2
